# revision 32
# baseline (speedup 1.0000x reference)
"""AttentionalEmbed TRN2 kernel (8 NeuronCores).

Math (reference):
    scores = q @ g.T            [Q, G]
    s      = scores @ Y         [Q, G]
    attn   = softmax(s, -1)
    r      = attn @ g           [Q, D]
    out    = tanh([q, r] @ W.T + b)

Key transforms:
  1. Associativity: s = q @ (g.T @ Y) = q @ M with M = [D, G] — collapses the
     dominant matmul chain from O(Q*G*G) to O(D*G*G + Q*D*G) FLOPs (8x fewer).
  2. Softmax row-constant invariance: softmax(q @ (g.T @ (Y - 0.5))) ==
     softmax(q @ (g.T @ Y)) — g.T @ (0.5*ones) adds a per-row constant which
     softmax ignores.  Centering Y halves all logit magnitudes and rounding
     errors.
  3. fp16 compute everywhere with fp32 PSUM accumulation (products of fp16
     are exact in fp32).  Measured end-to-end rel err vs fp32 ref: ~8e-3.

Sharding (8 cores):
  Phase A (M = g.T @ Y0): Y columns sharded; g replicated.
  Phase B (attention + head): queries sharded; M, g, W replicated.
"""

import numpy as np

import concourse.bass as bass
import concourse.mybir as mybir
import concourse.tile as tile
from concourse import bacc
from concourse.bass_utils import run_bass_kernel_spmd
from concourse.masks import make_identity

F16 = mybir.dt.float16
F32 = mybir.dt.float32

Q, G, D, OUT = 8192, 8192, 512, 512
NCORES = 8
QC = Q // NCORES      # queries per core
KC = G // NCORES      # gallery-label columns per core (phase A shard)


# --------------------------------------------------------------------------
# Phase A: M_shard = g.T @ Y0_shard   ([D, KC] = [G, D].T @ [G, KC]), fp16
# --------------------------------------------------------------------------
def build_phase_a(g_rows=G, d_dim=D, kc=KC):
    nc = bacc.Bacc("TRN2", target_bir_lowering=False, debug=False,
                   num_devices=NCORES)
    d_g = nc.dram_tensor("g16", [g_rows, d_dim], F16, kind="ExternalInput")
    d_y = nc.dram_tensor("y0", [g_rows, kc], F16, kind="ExternalInput")
    d_m = nc.dram_tensor("m16", [d_dim, kc], F16, kind="ExternalOutput")

    j_chunks = g_rows // 128
    d_chunks = d_dim // 128
    k_tiles = kc // 512
    assert d_chunks * k_tiles <= 8

    with tile.TileContext(nc) as tc:
        with (
            tc.tile_pool(name="resa", bufs=1) as resa,
            tc.tile_pool(name="ev", bufs=4) as evp,
            tc.tile_pool(name="psa", bufs=1, space="PSUM") as psa,
        ):
            # load g and y0 fully resident, j-interleaved so j=0 arrives first
            g_sb, y_sb = [], []
            for j in range(j_chunks):
                gt = resa.tile([128, d_dim], F16, tag=f"g{j}", name=f"g{j}")
                yt = resa.tile([128, kc], F16, tag=f"y{j}", name=f"y{j}")
                nc.sync.dma_start(out=gt, in_=d_g[j * 128:(j + 1) * 128, :])
                nc.sync.dma_start(out=yt, in_=d_y[j * 128:(j + 1) * 128, :])
                g_sb.append(gt)
                y_sb.append(yt)
            # one k-tile group per pass: group kk's evacuation overlaps
            # group kk+1's matmul stream
            for kk in range(k_tiles):
                ps = {}
                for dd in range(d_chunks):
                    ps[dd] = psa.tile([128, 512], F32, tag=f"a{dd}_{kk % 2}",
                                      name=f"psa{dd}_{kk}")
                for j in range(j_chunks):
                    for dd in range(d_chunks):
                        nc.tensor.matmul(
                            ps[dd], g_sb[j][:, dd * 128:(dd + 1) * 128],
                            y_sb[j][:, kk * 512:(kk + 1) * 512],
                            start=(j == 0), stop=(j == j_chunks - 1),
                        )
                for dd in range(d_chunks):
                    ev = evp.tile([128, 512], F16, tag="ev")
                    nc.scalar.activation(ev, ps[dd],
                                         mybir.ActivationFunctionType.Copy)
                    nc.sync.dma_start(
                        out=d_m[dd * 128:(dd + 1) * 128,
                                kk * 512:(kk + 1) * 512],
                        in_=ev)
    nc.compile()
    return nc


# --------------------------------------------------------------------------
# Phase B: per-core flash-softmax attention over M, fp16 compute
#   sT-chunk = qT.T @ M (pairs of 512 cols, psum) -> exp with pairwise max,
#   deferred correction -> p (fp16, [i,k]) -> DMA-transpose -> pT ->
#   PV: r[i,:] = sum_kc pT[kc].T @ g[kc,:]  -> outT = tanh(W.T.T@[qT;rT]+b)
# --------------------------------------------------------------------------
def build_phase_b(g_rows=G, d_dim=D, qc=QC, out_dim=OUT):
    nc = bacc.Bacc("TRN2", target_bir_lowering=False, debug=False,
                   num_devices=NCORES)
    d_m = nc.dram_tensor("m16", [d_dim, g_rows], F16, kind="ExternalInput")
    d_qt = nc.dram_tensor("qt16", [d_dim, qc], F16, kind="ExternalInput")
    d_g = nc.dram_tensor("g16", [g_rows, d_dim], F16, kind="ExternalInput")
    d_wt = nc.dram_tensor("wt16", [2 * d_dim, out_dim], F16,
                          kind="ExternalInput")
    d_b = nc.dram_tensor("bias", [out_dim, 1], F32, kind="ExternalInput")
    d_o = nc.dram_tensor("outt", [out_dim, qc], F32, kind="ExternalOutput")

    d_chunks = d_dim // 128          # 4
    IT_W = min(512, qc)              # i-tile width for PV/final
    i_tiles = qc // IT_W             # 2
    ic_per_it = IT_W // 128          # 4
    groups = g_rows // 512           # 16 (one 512-col psum bank each)
    k_chunks = g_rows // 128         # 64
    f_chunks = 2 * d_dim // 128      # 8
    o_chunks = out_dim // 128        # 4

    EXP = mybir.ActivationFunctionType.Exp
    TANH = mybir.ActivationFunctionType.Tanh
    COPY = mybir.ActivationFunctionType.Copy

    with tile.TileContext(nc) as tc:
        with (
            tc.tile_pool(name="res", bufs=1) as res,
            tc.tile_pool(name="pp", bufs=2) as pp,
            tc.tile_pool(name="ptp", bufs=1) as ptp,
            tc.tile_pool(name="gpv", bufs=6) as gpv,
            tc.tile_pool(name="rt", bufs=2) as rtp,
            tc.tile_pool(name="ot", bufs=2) as otp,
            tc.tile_pool(name="st", bufs=4) as st,
            tc.tile_pool(name="pss", bufs=4, space="PSUM") as pss,
            tc.tile_pool(name="psr", bufs=1, space="PSUM") as psr,
        ):
            # ---- resident tensors (m16 loaded in column chunks so the
            # s-stage can start before the whole matrix arrives)
            qt_sb = []
            for dd in range(d_chunks):
                t = res.tile([128, qc], F16, tag=f"qt{dd}", name=f"qt{dd}")
                nc.sync.dma_start(out=t, in_=d_qt[dd * 128:(dd + 1) * 128, :])
                qt_sb.append(t)
            m_sb = [res.tile([128, g_rows], F16, tag=f"m{dd}", name=f"m{dd}")
                    for dd in range(d_chunks)]
            m_load_chunk = 1024
            for cc in range(g_rows // m_load_chunk):
                for dd in range(d_chunks):
                    nc.sync.dma_start(
                        out=m_sb[dd][:, cc * m_load_chunk:
                                     (cc + 1) * m_load_chunk],
                        in_=d_m[dd * 128:(dd + 1) * 128,
                                cc * m_load_chunk:(cc + 1) * m_load_chunk])
            wt_sb = []
            for ff in range(f_chunks):
                t = res.tile([128, out_dim], F16, tag=f"wt{ff}", name=f"wt{ff}")
                nc.sync.dma_start(out=t, in_=d_wt[ff * 128:(ff + 1) * 128, :])
                wt_sb.append(t)
            b_sb = []
            for oo in range(o_chunks):
                t = res.tile([128, 1], F32, tag=f"b{oo}", name=f"b{oo}")
                nc.sync.dma_start(out=t, in_=d_b[oo * 128:(oo + 1) * 128, :])
                b_sb.append(t)
            ident = res.tile([128, 128], F16, tag="ident")
            make_identity(nc, ident[:])

            for it in range(i_tiles):
                pt = ptp.tile([128, k_chunks, IT_W], F16, tag="pt")
                for ic in range(ic_per_it):
                    i = it * ic_per_it + ic
                    p_t = pp.tile([128, g_rows], F16, tag="p")
                    nmp = st.tile([128, groups], F32, tag="nmp")
                    lt = st.tile([128, groups], F32, tag="lt")
                    # ---- s-stage: 512-wide k groups, 4 psum slots
                    for h in range(groups):
                        ps_s = pss.tile([128, 512], F32, tag="s")
                        for dd in range(d_chunks):
                            lhsT = qt_sb[dd][:, i * 128:(i + 1) * 128]
                            nc.tensor.matmul(
                                ps_s, lhsT,
                                m_sb[dd][:, h * 512:(h + 1) * 512],
                                start=(dd == 0), stop=(dd == d_chunks - 1),
                            )
                        nc.vector.reduce_max(out=nmp[:, h:h + 1], in_=ps_s,
                                             axis=mybir.AxisListType.X,
                                             negate=True)
                        nc.scalar.activation(
                            p_t[:, h * 512:(h + 1) * 512], ps_s, EXP,
                            bias=nmp[:, h:h + 1],
                            accum_out=lt[:, h:h + 1])
                    # ---- softmax correction across pairs
                    negm = st.tile([128, 1], F32, tag="negm")
                    nc.vector.tensor_reduce(out=negm, in_=nmp,
                                            op=mybir.AluOpType.min,
                                            axis=mybir.AxisListType.X)
                    cpair = st.tile([128, groups], F32, tag="cpair")
                    nc.scalar.activation(cpair, nmp, EXP, bias=negm,
                                         scale=-1.0)
                    ltmp = st.tile([128, groups], F32, tag="ltmp")
                    nc.vector.tensor_mul(ltmp, lt, cpair)
                    lsum = st.tile([128, 1], F32, tag="lsum")
                    nc.vector.reduce_sum(out=lsum, in_=ltmp,
                                         axis=mybir.AxisListType.X)
                    rinv = st.tile([128, 1], F32, tag="rinv")
                    nc.vector.reciprocal(rinv, lsum)
                    spair = st.tile([128, groups], F32, tag="spair")
                    nc.vector.tensor_scalar_mul(spair, cpair, rinv)
                    for h in range(groups):
                        # scaled transpose: p_blk.T @ diag(spair[:, h]) both
                        # transposes the block and applies the softmax
                        # correction/normalization in one PE pass
                        diag_t = st.tile([128, 128], F16, tag=f"diag{h % 2}",
                                         name="diag_t")
                        nc.vector.tensor_scalar_mul(diag_t, ident,
                                                    spair[:, h:h + 1])
                        ps_t = psr.tile([128, 4, 128], F32, tag=f"r{h % 4}",
                                        name="ps_t")
                        for u in range(4):
                            kc_ = h * 4 + u
                            nc.tensor.matmul(
                                ps_t[:, u, :],
                                p_t[:, kc_ * 128:(kc_ + 1) * 128], diag_t,
                                start=True, stop=True)
                        nc.any.tensor_copy(
                            out=pt[:, h * 4:(h + 1) * 4,
                                   ic * 128:(ic + 1) * 128],
                            in_=ps_t)
                # ---- PV: r[i_chunk] = sum_kc pT[kc].T @ g[kc, :]
                ps_pv = [psr.tile([128, d_dim], F32, tag=f"r{u}",
                                  name=f"pspv{u}") for u in range(ic_per_it)]
                for kc_ in range(k_chunks):
                    g_t = gpv.tile([128, d_dim], F16, tag="gpv")
                    nc.sync.dma_start(
                        out=g_t, in_=d_g[kc_ * 128:(kc_ + 1) * 128, :])
                    for u in range(ic_per_it):
                        nc.tensor.matmul(
                            ps_pv[u], pt[:, kc_, u * 128:(u + 1) * 128], g_t,
                            start=(kc_ == 0), stop=(kc_ == k_chunks - 1),
                        )
                # r16[u]: [128 i, 512 d] fp16; then DMA-transpose to rtT
                rtT = rtp.tile([128, d_chunks, IT_W], F16, tag="rtT", bufs=1)
                for u in range(ic_per_it):
                    r16 = rtp.tile([128, d_dim], F16, tag="r16")
                    nc.scalar.activation(r16, ps_pv[u], COPY)
                    ps_t2 = psr.tile([128, d_chunks, 128], F16, tag=f"r{u}",
                                     name="ps_t2")
                    for dd in range(d_chunks):
                        nc.tensor.transpose(
                            ps_t2[:, dd, :],
                            r16[:, dd * 128:(dd + 1) * 128], ident)
                    nc.any.tensor_copy(
                        out=rtT[:, :, u * 128:(u + 1) * 128], in_=ps_t2)
                # ---- final: outT[o, :] = tanh(sum_f wt[f].T @ xT[f] + b)
                for oo in range(o_chunks):
                    ps_o = pss.tile([128, IT_W], F32, tag="s", name="ps_o")
                    for ff in range(f_chunks):
                        if ff < d_chunks:
                            rhs = qt_sb[ff][:, it * IT_W:(it + 1) * IT_W]
                        else:
                            rhs = rtT[:, ff - d_chunks, :]
                        nc.tensor.matmul(
                            ps_o, wt_sb[ff][:, oo * 128:(oo + 1) * 128], rhs,
                            start=(ff == 0), stop=(ff == f_chunks - 1),
                        )
                    o_t = otp.tile([128, IT_W], F32, tag="ot")
                    nc.scalar.activation(o_t, ps_o, TANH, bias=b_sb[oo])
                    nc.sync.dma_start(
                        out=d_o[oo * 128:(oo + 1) * 128,
                                it * IT_W:(it + 1) * IT_W],
                        in_=o_t)
    nc.compile()
    return nc


_CACHE = {}


def _get(name, builder):
    if name not in _CACHE:
        _CACHE[name] = builder()
    return _CACHE[name]


def kernel(query_encode, gallery_encode, gallery_label, W, b):
    q = np.asarray(query_encode, np.float32)
    g = np.asarray(gallery_encode, np.float32)
    Y = np.asarray(gallery_label, np.float32)
    Wm = np.asarray(W, np.float32)
    bv = np.asarray(b, np.float32)

    g16 = g.astype(np.float16)

    # ---- phase A: M = g.T @ (Y - 0.5), column-sharded over cores
    nc_a = _get("a", build_phase_a)
    Y016 = (Y - np.float32(0.5)).astype(np.float16)
    in_a = []
    for c in range(NCORES):
        in_a.append({
            "g16": g16,
            "y0": np.ascontiguousarray(Y016[:, c * KC:(c + 1) * KC]),
        })
    res_a = run_bass_kernel_spmd(nc_a, in_a, core_ids=list(range(NCORES)))
    M16 = np.concatenate([res_a.results[c]["m16"] for c in range(NCORES)],
                         axis=1)  # [D, G] fp16

    # ---- phase B: queries sharded over cores
    nc_b = _get("b", build_phase_b)
    qt16 = np.ascontiguousarray(q.T.astype(np.float16))      # [D, Q]
    wt16 = np.ascontiguousarray(Wm.T.astype(np.float16))     # [2D, OUT]
    b2 = np.ascontiguousarray(bv.reshape(OUT, 1))
    in_b = []
    for c in range(NCORES):
        in_b.append({
            "m16": M16,
            "qt16": np.ascontiguousarray(qt16[:, c * QC:(c + 1) * QC]),
            "g16": g16,
            "wt16": wt16,
            "bias": b2,
        })
    res_b = run_bass_kernel_spmd(nc_b, in_b, core_ids=list(range(NCORES)))
    out = np.concatenate(
        [res_b.results[c]["outt"].T for c in range(NCORES)], axis=0)
    return np.ascontiguousarray(out.astype(np.float32))
